# revision 1
# baseline (speedup 1.0000x reference)
"""Channel-wise tensor product (e3nn-style) Trainium2 Bass kernel.

out[n] = concat(o0, o1, o2, o3, o4) with
  o0[u]      = w0[u] * s0[u] * y0
  o1[u,k]    = w1[u] * s0[u] * y1[k]
  o2[u,i]    = w2[u] * s1[u,i] * y0
  o3[u]      = w3[u]/sqrt(3) * sum_i s1[u,i] y1[i]
  o4[u,k]    = w4[u]/sqrt(2) * (s1[u,:] x y1)[k]

Sharding: pure data parallel over the batch dim across 8 cores.
Layout: batch rows on SBUF partitions (128-row tiles), channels on the
free dim. Per-row scalars y0/y1 are per-partition scalar operands; the
per-channel weights are replicated across partitions host-side with CG
normalization folded in.

Engine split per 128-row tile:
  ACT    3x broadcast products P_j = X * y1_j (FD 512) + Q0 = s0*y0
  PE     path-4 cross differences as +-identity matmuls into PSUM
         (exact fp32 for +-1 weights)
  GPSIMD path-3 sum E = d0+d1+d2 (SBUF tensor-tensor adds)
  DVE    all five output-segment weight multiplies (single O writer)
  SP     one input DMA + one output DMA per tile
"""

import numpy as np

import concourse.bass as bass
import concourse.tile as tile
from concourse import bacc, mybir
from concourse.bass_utils import run_bass_kernel_spmd

N_CORES = 8
B = 65536
U = 128
ROWS = B // N_CORES          # 8192 rows per core
NT = ROWS // 128             # 64 tiles of 128 rows
SQRT2 = 1.4142135623730951
SQRT3 = 1.7320508075688772

F32 = mybir.dt.float32
MUL = mybir.AluOpType.mult
ADD = mybir.AluOpType.add
SUB = mybir.AluOpType.subtract
COPY = mybir.ActivationFunctionType.Copy


def build_nc() -> bass.Bass:
    nc = bacc.Bacc("TRN2", target_bir_lowering=False, debug=False)

    x1s = nc.dram_tensor("x1s", (ROWS, 4 * U), F32, kind="ExternalInput").ap()
    x2s = nc.dram_tensor("x2s", (128, 4 * NT), F32, kind="ExternalInput").ap()
    # 11*U cols mirror the output layout; 3*U extra cols hold w1 in k-slab
    # order so o1's in1 is unit-stride.
    wbig = nc.dram_tensor("wbig", (128, 14 * U), F32, kind="ExternalInput").ap()
    eye2 = nc.dram_tensor("eye2", (128, 2 * U), F32, kind="ExternalInput").ap()
    out = nc.dram_tensor("out", (ROWS, 11 * U), F32, kind="ExternalOutput").ap()

    with tile.TileContext(nc) as tc:
        with (
            tc.tile_pool(name="const", bufs=1) as cpool,
            tc.tile_pool(name="xin", bufs=10) as xpool,
            tc.tile_pool(name="prod", bufs=6) as ppool,
            tc.tile_pool(name="smal", bufs=4) as spool,
            tc.tile_pool(name="outp", bufs=8) as opool,
            tc.tile_pool(name="psum", bufs=4, space="PSUM") as pspool,
        ):
            WB = cpool.tile([128, 14 * U], F32)
            nc.sync.dma_start(WB[:], wbig[:])
            X2 = cpool.tile([128, 4 * NT], F32)
            nc.sync.dma_start(X2[:], x2s[:])
            EYE = cpool.tile([128, 2 * U], F32)
            nc.sync.dma_start(EYE[:], eye2[:])
            IPOS = EYE[:, 0:U]
            INEG = EYE[:, U:2 * U]

            # Input DMAs are prefetched PF tiles ahead so the Sync queue
            # issues in(t+PF) before stalling on out(t)'s wait — otherwise
            # every tile pays the full load->compute->store chain latency.
            PF = 8
            xtiles = {}

            def load_x(t):
                X = xpool.tile([128, 4 * U], F32)
                nc.sync.dma_start(X[:], x1s[t * 128:(t + 1) * 128, :])
                xtiles[t] = X

            for t in range(PF):
                load_x(t)

            for t in range(NT):
                if t + PF < NT:
                    load_x(t + PF)
                X = xtiles.pop(t)

                y0 = X2[:, 4 * t:4 * t + 1]

                # P_j = X * y1_j, each (128, 512): cols 0:128 are path-1's
                # a_j = s0*y1_j; cols 128:512 hold the s1*y1_j products.
                P = ppool.tile([128, 3 * 4 * U], F32)
                for j in range(3):
                    yj = X2[:, 4 * t + 1 + j:4 * t + 2 + j]
                    nc.scalar.activation(
                        P[:, j * 512:(j + 1) * 512], X[:], COPY, scale=yj
                    )

                O = opool.tile([128, 11 * U], F32)

                # path 0: o0 = (s0 * y0) * w0 fused on DVE (DVE keeps O
                # single-writer so the output DMA needs only one wait)
                nc.vector.scalar_tensor_tensor(
                    O[:, 0:U], X[:, 0:U], y0, WB[:, 0:U], MUL, MUL
                )

                # path 2: o2 = (s1 * y0) * w2
                nc.vector.scalar_tensor_tensor(
                    O[:, 4 * U:7 * U], X[:, U:4 * U], y0, WB[:, 4 * U:7 * U],
                    MUL, MUL,
                )

                # path 1 on DVE: o1[u,k] = a_k[u] * w1[u]; dest col 128+3u+k
                o1dst = O[:, U:4 * U].rearrange("p (u k) -> p k u", k=3)
                a_src = P[:].rearrange("p (j s) -> p j s", j=3)[:, :, 0:U]
                w1v = WB[:, 11 * U:14 * U].rearrange("p (k u) -> p k u", k=3)
                nc.vector.tensor_tensor(o1dst, a_src, w1v, MUL)

                # path 3 on GPSIMD: E = d0 + d1 + d2 (diag products from P)
                E = spool.tile([128, U], F32)
                d0 = P[:, 512 * 0 + U + 0:512 * 0 + 4 * U:3]
                d1 = P[:, 512 * 1 + U + 1:512 * 1 + 4 * U:3]
                d2 = P[:, 512 * 2 + U + 2:512 * 2 + 4 * U:3]
                nc.gpsimd.tensor_tensor(E[:], d0, d1, ADD)
                nc.gpsimd.tensor_tensor(E[:], E[:], d2, ADD)
                # o3 = E * w3'
                nc.vector.tensor_tensor(
                    O[:, 7 * U:8 * U], E[:], WB[:, 7 * U:8 * U], MUL
                )

                # path 4 cross differences on PE into PSUM (k-slab layout):
                #   F[:, k*U+u] = c[k+1,k+2] - c[k+2,k+1]
                F = pspool.tile([128, 3 * U], F32)
                for k in range(3):
                    i1, j1 = (k + 1) % 3, (k + 2) % 3
                    i2, j2 = (k + 2) % 3, (k + 1) % 3
                    a = P[:, 512 * j1 + U + i1:512 * j1 + 4 * U:3]
                    b = P[:, 512 * j2 + U + i2:512 * j2 + 4 * U:3]
                    fdst = F[:, k * U:(k + 1) * U]
                    nc.tensor.matmul(fdst, IPOS, a, start=True, stop=False)
                    nc.tensor.matmul(fdst, INEG, b, start=False, stop=True)
                # o4[u,k] = F_k[u] * w4'[u]; dest col 1024+3u+k
                o4dst = O[:, 8 * U:11 * U].rearrange("p (u k) -> p k u", k=3)
                fsrc = F[:].rearrange("p (k u) -> p k u", k=3)
                w4v = WB[:, 8 * U:11 * U].rearrange("p (k u) -> p k u", k=3)
                nc.vector.tensor_tensor(o4dst, fsrc, w4v, MUL)

                nc.sync.dma_start(out[t * 128:(t + 1) * 128, :], O[:])

    nc.compile()
    return nc


def _host_prep(x1, x2, weight):
    """Shard x1/x2 per core; build the replicated weight layout."""
    x1 = np.ascontiguousarray(x1, dtype=np.float32)
    x2 = np.ascontiguousarray(x2, dtype=np.float32)
    w = np.asarray(weight, dtype=np.float32).reshape(5, U)

    wrow = np.concatenate([
        w[0],
        np.repeat(w[1], 3),          # o1 interleaved: col 3u+k -> w1[u]
        np.repeat(w[2], 3),          # o2 interleaved
        w[3] / SQRT3,
        np.tile(w[4] / SQRT2, 3),    # o4 k-slab: col k*U+u -> w4'[u]
        np.tile(w[1], 3),            # o1 in1, k-slab (unit stride)
    ])
    wbig = np.ascontiguousarray(np.broadcast_to(wrow, (128, 14 * U)))

    # +I / -I for the PE pre-sums. -I scales the subtracted cross term.
    eye = np.eye(U, dtype=np.float32)
    eye2 = np.ascontiguousarray(np.concatenate([eye, -eye], axis=1))
    eye2 = np.ascontiguousarray(eye2)

    in_maps = []
    for c in range(N_CORES):
        x1c = x1[c * ROWS:(c + 1) * ROWS]
        x2c = x2[c * ROWS:(c + 1) * ROWS]
        # x2s[p, 4t+c] = x2c[t*128+p, c]
        x2c = np.ascontiguousarray(
            x2c.reshape(NT, 128, 4).transpose(1, 0, 2).reshape(128, 4 * NT)
        )
        in_maps.append({"x1s": x1c, "x2s": x2c, "wbig": wbig, "eye2": eye2})
    return in_maps


_NC_CACHE = {}


def _ensure_ntff_hook():
    """The agent image lacks antenv.axon_hooks; synthesize it so
    run_bass_kernel_spmd(trace=True) can register the NTFF profiler."""
    import sys
    import types

    try:
        import antenv.axon_hooks  # noqa: F401
        return
    except ImportError:
        pass
    mod = types.ModuleType("antenv.axon_hooks")
    state = {"hook": None}

    def set_axon_ntff_profile_hook(hook):
        state["hook"] = hook

    def get_axon_ntff_profile_hook():
        if state["hook"] is None:
            import os

            so = "/opt/axon/libaxon_pjrt.so"
            if os.path.exists(so):
                try:
                    from trn_agent_boot.trn_boot import _ntff_profile_via_ctypes

                    state["hook"] = _ntff_profile_via_ctypes(so)
                except Exception:
                    state["hook"] = None
        return state["hook"]

    mod.set_axon_ntff_profile_hook = set_axon_ntff_profile_hook
    mod.get_axon_ntff_profile_hook = get_axon_ntff_profile_hook
    sys.modules["antenv.axon_hooks"] = mod


def kernel(x1, x2, weight, trace=False):
    assert x1.shape == (B, 4 * U) and x2.shape == (B, 4)
    if trace:
        _ensure_ntff_hook()
    in_maps = _host_prep(x1, x2, weight)
    if "nc" not in _NC_CACHE:
        _NC_CACHE["nc"] = build_nc()
    nc = _NC_CACHE["nc"]
    res = run_bass_kernel_spmd(
        nc, in_maps, core_ids=list(range(N_CORES)), trace=trace
    )
    out = np.concatenate([res.results[c]["out"] for c in range(N_CORES)], axis=0)
    if trace:
        kernel.last_exec_time_ns = res.exec_time_ns
        kernel.last_results = res
    return out



# revision 8
# speedup vs baseline: 1.0048x; 1.0048x over previous
"""Channel-wise tensor product (e3nn-style) Trainium2 Bass kernel.

out[n] = concat(o0, o1, o2, o3, o4) with
  o0[u]      = w0[u] * s0[u] * y0
  o1[u,k]    = w1[u] * s0[u] * y1[k]
  o2[u,i]    = w2[u] * s1[u,i] * y0
  o3[u]      = w3[u]/sqrt(3) * sum_i s1[u,i] y1[i]
  o4[u,k]    = w4[u]/sqrt(2) * (s1[u,:] x y1)[k]

Sharding: pure data parallel over the batch dim across 8 cores.
Layout: batch rows on SBUF partitions (128-row tiles), channels on the
free dim. fp16 end-to-end I/O halves HBM traffic vs f32 (the rel-err
budget of 2e-2 admits ~1e-3 fp16 error with big margin).

Weight folding: the three broadcast products P_j = (X * y1_j) * W_j use
a per-j weight vector W_j = [w1 | (i==j ? w3' : w4')] so every term
lands pre-weighted: s0 slabs carry w1 (path 1), diagonal i==j s1 terms
carry w3' (path 3), off-diagonal carry w4' (path 4). The PE then only
needs +-identity matmuls to sum/difference slabs into PSUM, and the
ACT engine cast-copies PSUM f32 -> fp16 into the interleaved (u,k)
output layout. Paths 0/2 are direct DVE STT writes into O.

Engine split per 128-row tile:
  DVE    3x STT products P_j (FD 512, fp16 2x mode) + o0/o2 STT
  PE     +I/-I matmuls: o1 gather (FD 384), o3 diag-sum (3x128),
         o4 cross +-sums (6x128), all into one PSUM tile
  ACT    3 interleaving cast-copies PSUM->O (fp16)
  SP     one input DMA + one output DMA per tile
"""

import numpy as np

import concourse.bass as bass
import concourse.tile as tile
from concourse import bacc, mybir
from concourse.bass_utils import run_bass_kernel_spmd

N_CORES = 8
B = 65536
U = 128
ROWS = B // N_CORES          # 8192 rows per core
NT = ROWS // 128             # 64 tiles of 128 rows
SQRT2 = 1.4142135623730951
SQRT3 = 1.7320508075688772

F16 = mybir.dt.float16
F32 = mybir.dt.float32
MUL = mybir.AluOpType.mult
COPY = mybir.ActivationFunctionType.Copy


def build_nc() -> bass.Bass:
    nc = bacc.Bacc("TRN2", target_bir_lowering=False, debug=False)

    x1s = nc.dram_tensor("x1s", (ROWS, 4 * U), F16, kind="ExternalInput").ap()
    x2s = nc.dram_tensor("x2s", (128, 4 * NT), F32, kind="ExternalInput").ap()
    # per-j folded weights for the products: wj[j] = [w1 | i==j?w3':w4']
    wj = nc.dram_tensor("wj", (128, 3 * 4 * U), F16, kind="ExternalInput").ap()
    # [w0 | repeat(w2,3)] for the o0/o2 direct STT writes
    wstt = nc.dram_tensor("wstt", (128, 4 * U), F16, kind="ExternalInput").ap()
    eye1 = nc.dram_tensor("eye1", (128, U), F16, kind="ExternalInput").ap()
    out = nc.dram_tensor("out", (ROWS, 11 * U), F16, kind="ExternalOutput").ap()

    with tile.TileContext(nc) as tc:
        with (
            tc.tile_pool(name="const", bufs=1) as cpool,
            tc.tile_pool(name="xin", bufs=12) as xpool,
            tc.tile_pool(name="prod", bufs=6) as ppool,
            tc.tile_pool(name="outp", bufs=8) as opool,
            tc.tile_pool(name="psum", bufs=4, space="PSUM") as pspool,
        ):
            WJ = cpool.tile([128, 3 * 4 * U], F16)
            nc.sync.dma_start(WJ[:], wj[:])
            WS = cpool.tile([128, 4 * U], F16)
            nc.sync.dma_start(WS[:], wstt[:])
            X2 = cpool.tile([128, 4 * NT], F32)
            nc.sync.dma_start(X2[:], x2s[:])
            EYE = cpool.tile([128, U], F16)
            nc.sync.dma_start(EYE[:], eye1[:])
            IPOS = EYE[:, 0:U]

            # Prefetch input DMAs PF tiles ahead so the Sync queue issues
            # in(t+PF) before stalling on out(t)'s wait.
            PF = 8
            xtiles = {}

            def load_x(t):
                X = xpool.tile([128, 4 * U], F16)
                nc.sync.dma_start(X[:], x1s[t * 128:(t + 1) * 128, :])
                xtiles[t] = X

            for t in range(PF):
                load_x(t)

            for t in range(NT):
                if t + PF < NT:
                    load_x(t + PF)
                X = xtiles.pop(t)

                y0 = X2[:, 4 * t:4 * t + 1]

                # P_j = (X * y1_j) * W_j, each (128, 512) fp16 unit-stride
                P = ppool.tile([128, 3 * 4 * U], F16)
                for j in range(3):
                    yj = X2[:, 4 * t + 1 + j:4 * t + 2 + j]
                    nc.vector.scalar_tensor_tensor(
                        P[:, j * 512:(j + 1) * 512], X[:], yj,
                        WJ[:, j * 512:(j + 1) * 512], MUL, MUL,
                    )

                O = opool.tile([128, 11 * U], F16)

                # path 0: o0 = (s0 * y0) * w0
                nc.vector.scalar_tensor_tensor(
                    O[:, 0:U], X[:, 0:U], y0, WS[:, 0:U], MUL, MUL
                )
                # path 2: o2 = (s1 * y0) * w2 (input already (u,i) interleaved)
                nc.vector.scalar_tensor_tensor(
                    O[:, 4 * U:7 * U], X[:, U:4 * U], y0, WS[:, U:4 * U],
                    MUL, MUL,
                )

                # PSUM tile: bank0 = [o1 k-slabs 0:384 | o3 384:512],
                # bank1 = [o4 k-slabs 512:896 | pad]
                F = pspool.tile([128, 1024], F32)

                # path 1: one +I matmul gathers the three w1*s0*y1_k slabs
                o1mov = P[:].rearrange("p (k c) -> p k c", k=3)[:, :, 0:U]
                nc.tensor.matmul(F[:, 0:384], IPOS, o1mov, start=True, stop=True)

                # path 3: accumulate the three w3'*s1_j*y1_j diagonal slabs
                for j in range(3):
                    dj = P[:, 512 * j + U + j:512 * j + 4 * U:3]
                    nc.tensor.matmul(
                        F[:, 384:512], IPOS, dj, start=(j == 0), stop=(j == 2)
                    )
                # path 4: o4_k = A_k - B_k; the minus sign is folded into
                # the W_j pattern ((i-j)%3==1 slots hold -w4'), so both
                # terms accumulate with the +I stationary and each PSUM
                # region's group opens and closes back-to-back.
                for k in range(3):
                    i1, j1 = (k + 1) % 3, (k + 2) % 3
                    i2, j2 = (k + 2) % 3, (k + 1) % 3
                    a = P[:, 512 * j1 + U + i1:512 * j1 + 4 * U:3]
                    b = P[:, 512 * j2 + U + i2:512 * j2 + 4 * U:3]
                    fdst = F[:, 512 + k * U:512 + (k + 1) * U]
                    nc.tensor.matmul(fdst, IPOS, a, start=True, stop=False)
                    nc.tensor.matmul(fdst, IPOS, b, start=False, stop=True)

                # ACT cast-copies PSUM f32 -> O fp16, interleaving k-slabs
                # into the (u,k) output layout via 3D access patterns.
                o1dst = O[:, U:4 * U].rearrange("p (u k) -> p u k", k=3)
                o1src = F[:, 0:384].rearrange("p (k u) -> p u k", k=3)
                nc.scalar.activation(o1dst, o1src, COPY)
                nc.scalar.activation(O[:, 7 * U:8 * U], F[:, 384:512], COPY)
                o4dst = O[:, 8 * U:11 * U].rearrange("p (u k) -> p u k", k=3)
                o4src = F[:, 512:896].rearrange("p (k u) -> p u k", k=3)
                nc.scalar.activation(o4dst, o4src, COPY)

                nc.sync.dma_start(out[t * 128:(t + 1) * 128, :], O[:])

    nc.compile()
    return nc


def _host_prep(x1, x2, weight):
    """Shard x1/x2 per core; build the folded fp16 weight layouts."""
    x1 = np.asarray(x1, dtype=np.float32)
    x2 = np.ascontiguousarray(x2, dtype=np.float32)
    w = np.asarray(weight, dtype=np.float32).reshape(5, U)

    w3p = w[3] / SQRT3
    w4p = w[4] / SQRT2
    # wj[j] = [w1 | pattern_j interleaved (u,i)] with
    # pattern_j[u,i] = w3' if i==j, +w4' if (i-j)%3==2, -w4' if (i-j)%3==1
    wj_row = np.empty(3 * 4 * U, dtype=np.float32)
    for j in range(3):
        seg = np.empty((U, 3), dtype=np.float32)
        seg[:, j] = w3p
        seg[:, (j + 2) % 3] = w4p
        seg[:, (j + 1) % 3] = -w4p
        wj_row[j * 512:j * 512 + U] = w[1]
        wj_row[j * 512 + U:(j + 1) * 512] = seg.reshape(-1)
    wj_full = np.broadcast_to(wj_row.astype(np.float16), (128, 3 * 4 * U))
    wj_full = np.ascontiguousarray(wj_full)

    wstt_row = np.concatenate([w[0], np.repeat(w[2], 3)]).astype(np.float16)
    wstt = np.ascontiguousarray(np.broadcast_to(wstt_row, (128, 4 * U)))

    eye1 = np.ascontiguousarray(np.eye(U, dtype=np.float16))

    x1h = x1.astype(np.float16)

    in_maps = []
    for c in range(N_CORES):
        x1c = np.ascontiguousarray(x1h[c * ROWS:(c + 1) * ROWS])
        x2c = x2[c * ROWS:(c + 1) * ROWS]
        # x2s[p, 4t+c] = x2c[t*128+p, c]
        x2c = np.ascontiguousarray(
            x2c.reshape(NT, 128, 4).transpose(1, 0, 2).reshape(128, 4 * NT)
        )
        in_maps.append(
            {"x1s": x1c, "x2s": x2c, "wj": wj_full, "wstt": wstt, "eye1": eye1}
        )
    return in_maps


_NC_CACHE = {}


def _ensure_ntff_hook():
    """The agent image lacks antenv.axon_hooks; synthesize it so
    run_bass_kernel_spmd(trace=True) can register the NTFF profiler."""
    import sys
    import types

    try:
        import antenv.axon_hooks  # noqa: F401
        return
    except ImportError:
        pass
    mod = types.ModuleType("antenv.axon_hooks")
    state = {"hook": None}

    def set_axon_ntff_profile_hook(hook):
        state["hook"] = hook

    def get_axon_ntff_profile_hook():
        if state["hook"] is None:
            import os

            so = "/opt/axon/libaxon_pjrt.so"
            if os.path.exists(so):
                try:
                    from trn_agent_boot.trn_boot import _ntff_profile_via_ctypes

                    state["hook"] = _ntff_profile_via_ctypes(so)
                except Exception:
                    state["hook"] = None
        return state["hook"]

    mod.set_axon_ntff_profile_hook = set_axon_ntff_profile_hook
    mod.get_axon_ntff_profile_hook = get_axon_ntff_profile_hook
    sys.modules["antenv.axon_hooks"] = mod


def kernel(x1, x2, weight, trace=False):
    assert x1.shape == (B, 4 * U) and x2.shape == (B, 4)
    if trace:
        _ensure_ntff_hook()
    in_maps = _host_prep(x1, x2, weight)
    if "nc" not in _NC_CACHE:
        _NC_CACHE["nc"] = build_nc()
    nc = _NC_CACHE["nc"]
    res = run_bass_kernel_spmd(
        nc, in_maps, core_ids=list(range(N_CORES)), trace=trace
    )
    out = np.concatenate(
        [res.results[c]["out"].astype(np.float32) for c in range(N_CORES)],
        axis=0,
    )
    if trace:
        kernel.last_exec_time_ns = res.exec_time_ns
        kernel.last_results = res
    return out
